# revision 12
# baseline (speedup 1.0000x reference)
"""Channel-attention kernel for Trainium2 (8 NeuronCores, data-parallel over batch).

Math: the reference expands x (B,C,T,1) to 8 channels via a 1x1 conv before the
Q@K^T einsum.  Algebraically, with alpha=w1.w2, delta=b1.w2 and
s[b,c]=sum_t x[b,c,t]:

    energy[b,c,e] = alpha*G[b,c,e] + delta*s[b,e] + (terms const along e)
    G[b] = X[b] @ X[b]^T          (X[b] = x[b,:,:,0], shape (C,T))

Terms constant along the e (last) axis cancel in the min-max normalization;
only alpha*G + delta*s_e matters.  This cuts the contraction from T*8 down
to T (the advertised 8x headroom).

v11 design (lineage: v5 xbar-transpose 105us -> v6 PE-transpose 74.6 ->
v8-v10 copy/ring tuning ~69.3).  Key structural facts driving the layout:
  - the DMA-xbar TRANSPOSE mode hardware-serializes against ALL other
    DMA (known HW deadlock, Tile inserts drain waits), so X^T is built
    on the TensorEngine instead: matmul(lhsT=x_chunk, rhs=I) = chunk^T
    into PSUM at regular-matmul speed, then DVE/ACT copy to SBUF bf16
    (exact: bf16 values survive the f32 PSUM round trip).  DMA carries
    only the mandatory 8.2 MB loads + 4.1 MB stores (~34 us at the
    358 GB/s HBM limit), all on the sync/HWDGE ring, loads first.
  - only DVE and ACT can read PSUM (Pool has no port), so the
    PSUM->SBUF copy volume (XT + output chunks) is the throughput
    floor on those engines; copies are emitted widest-possible and
    alternate engines.
  - in-order engine FIFOs make ISSUE ORDER the schedule.  v10 traces
    showed the copy engines idling on the T->copy->Gram round trip
    (one pair deep) and the PE going HAM-cold in the gaps.  v11
    zippers Gram(p) 4-MM groups with T(p+1) 4-MM groups in one PE
    stream: the XT copies of pair p+1 land while Gram(p) streams, the
    PE stays dense (warm), and nothing waits a full phase.
  - out_sb has one buffer per pair (v10: pair p's first output copy
    waited on pair p-2's STORE completing), stores go out in quarters
    as soon as each half-chunk pair is copied.
  - residual is folded into the attention matmul: lhsT = I + gamma*A
    block-diagonal, so PSUM holds the final output chunk.
  - rank-1 (delta/alpha)*s_e update rides the end of the Gram PSUM
    accumulation group; row-sums come from cast accum_out.
"""

import numpy as np
import ml_dtypes
from contextlib import ExitStack

import concourse.bass as bass
import concourse.tile as tile
from concourse import mybir
from concourse.bass_utils import run_bass_kernel_spmd
from concourse.alu_op_type import AluOpType

F32 = mybir.dt.float32
BF16 = mybir.dt.bfloat16
AX = mybir.AxisListType.X

B, C, T = 64, 64, 4000
NCORES = 8
BPC = B // NCORES          # 8 batches per core
PAIRS = BPC // 2           # 4 pairs of 2 batches
ROWS = BPC * C             # 512 rows of (C,T) per core
TP = 4096                  # T padded to a multiple of 128
NKT = TP // 128            # 32 k-tiles
NG = NKT // 4              # 8 groups of 4 k-tiles
EPS = 1e-8
CSP = 1824                 # cast split: DVE (1.11 ns/col w/ accum) vs ACT


def _body(ctx, tc, out_ap, x_ap, idf_ap, alpha, doa, gamma):
    nc = tc.nc

    singles = ctx.enter_context(tc.tile_pool(name="singles", bufs=1))
    xfp = ctx.enter_context(tc.tile_pool(name="xfp", bufs=4))
    xtp = ctx.enter_context(tc.tile_pool(name="xtp", bufs=2))
    obp = ctx.enter_context(tc.tile_pool(name="obp", bufs=4))
    smalls = ctx.enter_context(tc.tile_pool(name="smalls", bufs=3))

    # PSUM budget (8 banks): ps_t 2, ps_g 2, ps_s 1, ps_o 3
    ps_t = ctx.enter_context(tc.tile_pool(name="ps_t", bufs=2, space="PSUM"))
    ps_s = ctx.enter_context(tc.tile_pool(name="ps_s", bufs=1, space="PSUM"))
    ps_g = ctx.enter_context(tc.tile_pool(name="ps_g", bufs=2, space="PSUM"))
    ps_o = ctx.enter_context(tc.tile_pool(name="ps_o", bufs=3, space="PSUM"))

    ident_f32 = singles.tile([128, 128], F32)
    ident_bf = singles.tile([128, 128], BF16)
    ones_row = singles.tile([1, 128], BF16)
    nc.vector.memset(ones_row[:], 1.0)
    # preload the ACT function tables during the ramp
    warm_act = singles.tile([1, 2], F32)
    nc.scalar.activation(
        warm_act[:], ones_row[0:1, 0:2], mybir.ActivationFunctionType.Exp
    )
    # persistent x_bf buffers (one per pair); pad zeroed once, here
    xbufs = [
        singles.tile([128, TP], BF16, name=f"xb{i}") for i in range(PAIRS)
    ]
    for xb in xbufs:
        nc.gpsimd.memset(xb[:, T:TP], 0.0)
    # persistent latt bufs (rotated p%2): off-diagonal blocks stay zero
    # forever, so each pair only writes its two diagonal 64-blocks
    lattbufs = [
        singles.tile([128, 128], BF16, name=f"la{i}") for i in range(2)
    ]
    for la in lattbufs:
        nc.gpsimd.memset(la[:], 0.0)

    st = [{} for _ in range(PAIRS)]

    # pair-0 load pieces: small head piece so the first cast + first
    # T-group start as early as possible
    P0L = (0, 512, CSP, 2912, T)

    def stL(p):
        """f32 loads on the sync/HWDGE ring (carries loads first, then
        stores, so loads stream back-to-back at HBM rate)."""
        v = st[p]
        rows = slice(p * 128, (p + 1) * 128)
        x_f32 = xfp.tile([128, T], F32)
        if p == 0:
            for q in range(4):
                lo, hi = P0L[q], P0L[q + 1]
                nc.sync.dma_start(x_f32[:, lo:hi], x_ap[rows, lo:hi])
                if q == 0:
                    # identity rides after the head piece
                    nc.sync.dma_start(ident_f32[:], idf_ap)
                    nc.vector.tensor_copy(ident_bf[:], ident_f32[:])
        else:
            nc.sync.dma_start(x_f32[:], x_ap[rows, :])
        v["x_f32"] = x_f32
        v["x_bf"] = xbufs[p]

    def stC(p):
        """casts with row-sum accumulation: DVE [0:CSP], ACT [CSP:T]."""
        v = st[p]
        x_f32, x_bf = v["x_f32"], v["x_bf"]
        s_ab = smalls.tile([128, 3], F32, tag="sab")
        if p == 0:
            nc.vector.tensor_scalar(
                x_bf[:, 0:512], x_f32[:, 0:512], scalar1=1.0, scalar2=0.0,
                op0=AluOpType.mult, op1=AluOpType.add, accum_out=s_ab[:, 0:1],
            )
            nc.vector.tensor_scalar(
                x_bf[:, 512:CSP], x_f32[:, 512:CSP], scalar1=1.0, scalar2=0.0,
                op0=AluOpType.mult, op1=AluOpType.add, accum_out=s_ab[:, 2:3],
            )
        else:
            nc.vector.tensor_scalar(
                x_bf[:, 0:CSP], x_f32[:, 0:CSP], scalar1=1.0, scalar2=0.0,
                op0=AluOpType.mult, op1=AluOpType.add, accum_out=s_ab[:, 0:1],
            )
            nc.vector.memset(s_ab[:, 2:3], 0.0)
        nc.scalar.activation(
            x_bf[:, CSP:T], x_f32[:, CSP:T],
            mybir.ActivationFunctionType.Copy, accum_out=s_ab[:, 1:2],
        )
        s_col = smalls.tile([128, 1], F32, tag="scol")
        nc.vector.tensor_reduce(s_col[:], s_ab[:], axis=AX, op=AluOpType.add)
        v["s_col"] = s_col

    def emit_tgroup(p, g):
        """one 4-chunk transpose group of pair p: 4 identity matmuls into
        a PSUM bank + one wide copy to xt (engines alternate by group)."""
        v = st[p]
        x_bf = v["x_bf"]
        if g == 0:
            v["xt"] = xtp.tile([128, TP], BF16, name=f"xt{p}")
        xt = v["xt"]
        ps = ps_t.tile([128, 512], F32, tag="t")
        for j in range(4):
            k = 4 * g + j
            nc.tensor.matmul(
                ps[:, j * 128:(j + 1) * 128],
                lhsT=x_bf[:, k * 128:(k + 1) * 128],
                rhs=ident_bf[:],
                start=True,
                stop=True,
            )
        base = g * 512
        if g % 2 == 0:
            nc.vector.tensor_copy(xt[:, base:base + 512], ps[:])
        else:
            nc.scalar.copy(xt[:, base:base + 512], ps[:])

    def emit_ghead(p, psum_g):
        """rank-1 (delta/alpha)*s_e update OPENS the Gram accumulation
        group (start=True), so the pair-boundary critical path ends on
        the last plain Gram matmul, not on the s-row chain."""
        v = st[p]
        st_ps = ps_s.tile([1, 128], F32, tag="st")
        nc.tensor.transpose(st_ps[:], v["s_col"][:], ident_f32[:])
        rhs_aux = smalls.tile([1, 128], BF16, tag="aux")
        nc.vector.tensor_scalar_mul(rhs_aux[:], st_ps[:], doa)
        nc.tensor.matmul(
            psum_g[:],
            lhsT=ones_row[:],
            rhs=rhs_aux[:],
            start=True,
            stop=False,
            skip_group_check=True,
        )
        v["psum_g"] = psum_g

    def emit_ggroup(p, g, psum_g):
        """one 4-MM Gram accumulation group of pair p."""
        xt = st[p]["xt"]
        for j in range(4):
            base = (4 * g + j) * 128
            nc.tensor.matmul(
                psum_g[:],
                lhsT=xt[:, base: base + 128],
                rhs=xt[:, base: base + 128],
                start=False,
                stop=(g == NG - 1 and j == 3),
                skip_group_check=True,
            )

    def stT0():
        for g in range(NG):
            emit_tgroup(0, g)

    def stT3x():
        """pair 3's X^T via the DMA xbar: all loads are done by the time
        cast3 lands, so the TRANSPOSE-mode serialization only delays
        early stores (which have slack) -- and it takes 4096 cols of
        PSUM-copy work off DVE/ACT in the tail window."""
        v = st[3]
        x_bf = v["x_bf"]
        xt = xtp.tile([128, TP], BF16, name="xt3")
        for q in range(2):
            lo, hi = q * (TP // 2), (q + 1) * (TP // 2)
            nc.sync.dma_start_transpose(
                xt[:, lo:hi].rearrange("q (k f) -> q k f", f=128),
                x_bf[:, lo:hi],
            )
        v["xt"] = xt

    def stGT(p):
        """zippered PE stream: Gram(p) group j alternating with T(p+1)
        group j -- pair p+1's XT copies land while Gram(p) streams, the
        PE never drains, and the T->copy->Gram round trip is amortized
        one pair ahead."""
        psum_g = ps_g.tile([128, 128], F32, tag="g")
        emit_ghead(p, psum_g)
        for g in range(NG):
            emit_ggroup(p, g, psum_g)
            emit_tgroup(p + 1, g)

    def stGx(p):
        """plain Gram for the last pair (no successor to zipper)."""
        psum_g = ps_g.tile([128, 128], F32, tag="g")
        emit_ghead(p, psum_g)
        for g in range(NG):
            emit_ggroup(p, g, psum_g)

    def stGy(p):
        """energy extraction + min-max softmax -> attention lhsT with the
        residual identity folded in (M = I + gamma*A, block-diagonal)."""
        v = st[p]
        psum_g = v["psum_g"]
        # Diagonal (64,64) blocks, scaled by alpha -> energy (128, 64)
        e_sb = smalls.tile([128, 64], F32, tag="esb")
        nc.vector.tensor_scalar_mul(e_sb[0:64, :], psum_g[0:64, 0:64], alpha)
        nc.vector.tensor_scalar_mul(
            e_sb[64:128, :], psum_g[64:128, 64:128], alpha
        )

        # min-max normalize along free axis, then softmax (normalized values
        # live in [0,1], so no max-subtraction is needed before exp)
        rmax = smalls.tile([128, 1], F32, tag="rmax")
        nc.vector.tensor_reduce(rmax[:], e_sb[:], axis=AX, op=AluOpType.max)
        rmin = smalls.tile([128, 1], F32, tag="rmin")
        nc.vector.tensor_reduce(rmin[:], e_sb[:], axis=AX, op=AluOpType.min)
        den = smalls.tile([128, 1], F32, tag="den")
        nc.vector.tensor_scalar(
            den[:], rmax[:], scalar1=rmin[:], scalar2=EPS,
            op0=AluOpType.subtract, op1=AluOpType.add,
        )
        rden = smalls.tile([128, 1], F32, tag="rden")
        nc.vector.reciprocal(rden[:], den[:])
        nbias = smalls.tile([128, 1], F32, tag="nbias")
        nc.vector.scalar_tensor_tensor(
            nbias[:], in0=rmin[:], scalar=-1.0, in1=rden[:],
            op0=AluOpType.mult, op1=AluOpType.mult,
        )
        ex = smalls.tile([128, 64], F32, tag="ex")
        nc.scalar.activation(
            ex[:], e_sb[:], mybir.ActivationFunctionType.Exp,
            bias=nbias[:], scale=rden[:],
        )
        ssum = smalls.tile([128, 1], F32, tag="ssum")
        nc.vector.tensor_reduce(ssum[:], ex[:], axis=AX, op=AluOpType.add)
        rsum = smalls.tile([128, 1], F32, tag="rsum")
        nc.vector.reciprocal(rsum[:], ssum[:])
        rsg = smalls.tile([128, 1], F32, tag="rsg")
        nc.vector.tensor_scalar_mul(rsg[:], rsum[:], gamma)

        # write gamma*A + I directly into the persistent latt's diagonal
        # 64-blocks (off-diagonal blocks are zero from the one-time memset)
        latt = lattbufs[p % 2]
        nc.vector.scalar_tensor_tensor(
            latt[0:64, 0:64], in0=ex[0:64, :], scalar=rsg[0:64],
            in1=ident_bf[0:64, 0:64], op0=AluOpType.mult, op1=AluOpType.add,
        )
        nc.vector.scalar_tensor_tensor(
            latt[64:128, 64:128], in0=ex[64:128, :], scalar=rsg[64:128],
            in1=ident_bf[64:128, 64:128], op0=AluOpType.mult,
            op1=AluOpType.add,
        )
        v["latt"] = latt

    def stA(p):
        """output chunks: PSUM holds the final result (residual folded
        into the matmul); bf16 copies alternate DVE/ACT; quarter-stores
        ride the sync ring behind the loads."""
        v = st[p]
        rows = slice(p * 128, (p + 1) * 128)
        x_bf, latt = v["x_bf"], v["latt"]
        out_sb = obp.tile([128, T], BF16)
        for ci in range(8):
            lo, hi = ci * 512, min((ci + 1) * 512, T)
            psum_o = ps_o.tile([128, hi - lo], F32, tag="o")
            nc.tensor.matmul(
                psum_o[:], lhsT=latt[:], rhs=x_bf[:, lo:hi],
                start=True, stop=True,
            )
            if ci in (0, 3, 6):
                nc.vector.tensor_copy(out_sb[:, lo:hi], psum_o[:])
            else:
                nc.scalar.copy(out_sb[:, lo:hi], psum_o[:])
            if ci % 2 == 1:
                qcols = slice(ci // 2 * 1024, hi)
                nc.sync.dma_start(out_ap[rows, qcols], out_sb[:, qcols])
        v.clear()

    sched = [
        (stL, 0), (stC, 0), (stL, 1), (stT0,),
        (stC, 1), (stL, 2), (stGT, 0), (stGy, 0),
        (stC, 2), (stL, 3), (stGT, 1), (stGy, 1),
        (stC, 3), (stT3x,),
        (stA, 0), (stGx, 2), (stGy, 2),
        (stA, 1), (stGx, 3), (stGy, 3),
        (stA, 2),
        (stA, 3),
    ]
    for fn, *args in sched:
        fn(*args)


_MODULE_CACHE = {}


def _build_module(alpha, doa, gamma):
    key = (alpha, doa, gamma)
    if key in _MODULE_CACHE:
        return _MODULE_CACHE[key]
    nc = bass.Bass(
        "TRN2", target_bir_lowering=False, debug=False, num_devices=NCORES
    )
    x_ap = nc.dram_tensor("x", (ROWS, T), F32, kind="ExternalInput").ap()
    idf_ap = nc.dram_tensor("idf", (128, 128), F32, kind="ExternalInput").ap()
    out_ap = nc.dram_tensor("out", (ROWS, T), BF16, kind="ExternalOutput").ap()
    with tile.TileContext(nc) as tc, ExitStack() as ctx:
        _body(ctx, tc, out_ap, x_ap, idf_ap, alpha, doa, gamma)
    if _LEGALIZE_WAITS:
        _split_waits(nc)
    _MODULE_CACHE[key] = nc
    return nc


# The wait-split legalization confuses CoreSim's bookkeeping (hand-built
# NoOps bypass nc.inst_map); tests flip this off for simulation runs.
_LEGALIZE_WAITS = True


def _split_waits(nc):
    """walrus TRN2 codegen allows only ONE sync wait per instruction; when
    Tile emits more (e.g. PSUM slot reuse: previous-writer completion +
    previous-reader), hoist the extras onto same-engine NoOps inserted
    immediately before — the sequencer dispatches in order, so the blocking
    semantics are identical."""
    nid = [0]
    for f in nc.m.functions:
        for block in f.blocks:
            out = []
            for inst in block.instructions:
                si = getattr(inst, "sync_info", None)
                if (
                    si is not None
                    and si.on_wait
                    and len(si.on_wait) > 1
                    and type(inst).__name__ != "InstNoOp"
                ):
                    waits = list(si.on_wait)
                    for w in waits[:-1]:
                        nid[0] += 1
                        out.append(
                            mybir.InstNoOp(
                                name=f"{inst.name}-wsplit{nid[0]}",
                                engine=inst.engine,
                                ins=[],
                                outs=[],
                                sync_info=mybir.SyncInfo(
                                    on_wait=[w], on_update=[]
                                ),
                                text_hint="wait-split",
                                bass_nofuse=True,
                            )
                        )
                    inst.sync_info = mybir.SyncInfo(
                        on_wait=waits[-1:], on_update=list(si.on_update)
                    )
                out.append(inst)
            block.instructions[:] = out


def _prepare(inputs):
    x = np.ascontiguousarray(
        np.asarray(inputs["x"], dtype=np.float32).reshape(B * C, T)
    )
    w1 = np.asarray(inputs["w1"], dtype=np.float64)
    b1 = np.asarray(inputs["b1"], dtype=np.float64)
    w2 = np.asarray(inputs["w2"], dtype=np.float64)
    b2 = np.asarray(inputs["b2"], dtype=np.float64)
    gamma = float(np.asarray(inputs["gamma"]))
    alpha = float(w1 @ w2)
    delta = float(b1 @ w2)
    assert abs(alpha) > 1e-12, "degenerate alpha not supported"
    nc = _build_module(alpha, delta / alpha, gamma)
    ident_f = np.eye(128, dtype=np.float32)
    in_maps = [
        {"x": x[i * ROWS:(i + 1) * ROWS], "idf": ident_f}
        for i in range(NCORES)
    ]
    return nc, in_maps


def kernel(**inputs):
    nc, in_maps = _prepare(inputs)
    res = run_bass_kernel_spmd(nc, in_maps, core_ids=list(range(NCORES)))
    out = np.concatenate([res.results[i]["out"] for i in range(NCORES)], axis=0)
    return out.astype(np.float32).reshape(B, C, T, 1)
